# revision 33
# baseline (speedup 1.0000x reference)
"""Trainium2 Bass kernel for nn_Attention_82540681494971.

Spatial self-attention block (LDM AttnBlock style, unscaled):
  qkv = conv1x1(x);  s = q^T k  [n x n] per (b,head);  attn = softmax(s, axis=-1)
  out[d,m] = sum_n v[d,n] attn[n,m];  y = conv1x1(out)

Shapes: B=4, C=64, H=W=64 -> n=4096 tokens, HEAD=4, d=16.

Sharding: 8 cores, core c handles batch b=c//2 and heads (0,1) if c%2==0
else (2,3). Each core computes a partial projection output over its two
heads' channels; host sums the two partials per batch and adds proj bias.

The kernel is ACT(exp)-bound: 2 heads x 4096 x 4096 exps per core at
1 elem/lane/cycle @ 1.2 GHz is ~220us. Everything else is organized to
keep the scalar engine streaming exp with minimal per-instruction
overhead and zero stalls:

- All-bf16 data path (inputs pre-cast on host). Scores s = q^T k with
  K=16 per head; error budget validated ~1.4e-3 vs gate 2e-2.
- Score matmuls rotate over 4 PE row strips (tile_position (32s, 0)) so
  LDWEIGHTS pulls ahead and up to 4 streams run concurrently.
- exp in 2 chunks of 2048 per n-tile (2 ACTIVATEs + 2 accumulator
  reads), double-buffered in ALL 8 PSUM banks ([128,2048] x 2).
- Projection is folded into AV: per n-tile, Wn = (v^T wp) * rinv
  [128 x 64] bf16; y[o,m] += Wn^T E chains with M=64. Chain PSUM lives
  *transiently* inside the score buffer that exp just drained (cols
  1536:2048, partition half 64*(mc%2)) - no dedicated AV banks needed.
- y partials accumulate in y_sb[128, 2048] (m-chunk mc at partition
  half mc%2, col block mc//2), DMA'd out per chunk at the end.
"""

import numpy as np
import ml_dtypes
from contextlib import ExitStack

import concourse.bass as bass
import concourse.mybir as mybir
import concourse.tile as tile
from concourse import bacc
from concourse.bass import ts, ds
from concourse.bass_utils import run_bass_kernel_spmd

F32 = mybir.dt.float32
BF16 = mybir.dt.bfloat16
AF = mybir.ActivationFunctionType

B, C, HEAD, D = 4, 64, 4, 16
N = 4096          # tokens = H*W
NT = 128          # n-tile (partition) size
NTILES = N // NT  # 32
MC = 512          # matmul free-dim chunk
MCN = N // MC     # 8 m-chunks
SCH = (1536, 1536, 1024)   # scores/exp PSUM chunking (2-buffer ring)
G = 4              # n-tiles per AV supergroup
NSG = NTILES // G  # supergroups per head


def _body(tc, y, x1, wq, wk, wv, wp):
    nc = tc.nc
    ctx = ExitStack()
    with ctx:
        pp = ctx.enter_context(tc.tile_pool(name="persist", bufs=1))
        cp = ctx.enter_context(tc.tile_pool(name="consts", bufs=1))

        # ---- constants (all bf16) ----
        wq_t = cp.tile([C + 1, 2 * D], BF16)
        wk_t = cp.tile([C + 1, 2 * D], BF16)
        wv_t = cp.tile([C + 1, 2 * D], BF16)
        wp_t = cp.tile([D, 2 * C], BF16)     # [16,128]: head0 cols 0-63, head1 64-127
        nc.gpsimd.dma_start(wq_t[:], wq[:])
        nc.gpsimd.dma_start(wk_t[:], wk[:])
        nc.gpsimd.dma_start(wv_t[:], wv[:])
        nc.gpsimd.dma_start(wp_t[:], wp[:])

        # ---- persistent SBUF ----
        # q/k replicated on 4 PE row strips (partitions 32s..32s+15),
        # head-major cols.
        qsp = pp.tile([96 + D, 2 * N], BF16)
        ksp = pp.tile([96 + D, 2 * N], BF16)
        v4_sb = pp.tile([96 + D, 2048], BF16)   # v, strip-local layout
        wp4_t = pp.tile([96 + D, 2 * C], BF16)  # wp replicated on 4 strips
        x1_t = pp.tile([C + 1, N], BF16)        # input (woven qkv reads it)
        # y partials: two partition-half partial sums (host adds them)
        y_sb = pp.tile([NT, N], F32)

        # ---- phase 0: k/q for head 0 ONLY (gates the first exp) ----
        # qsp/ksp use a strip-major column layout: chunk c of head h lives at
        # col4(h, c) = (c%4)*2048 + h*1024 + (c//4)*512. The (q|k, h, c)
        # matmul runs at PE column strip c%4 (output on PSUM partitions
        # 32s..32s+15) and evacuates straight into its own row strip of
        # qsp/ksp - no partition moves - then per-head [16,1024] DMAs
        # replicate each strip's block to the other strips. v and the
        # head-1 q/k are WOVEN into the main loop (see pending_misc).
        def qkv_mm(w_t, dst, h, c, ps):
            s = c % 4
            nc.tensor.matmul(
                ps[ds(32 * s, D), 0:MC], w_t[:, ts(h, D)],
                x1_t[:, ts(c, MC)],
                start=True, stop=True, tile_position=(0, 32 * s))
            if dst is v4_sb:
                col = h * 1024 + (c // 4) * MC
            else:
                col = h * 1024 + (c // 4) * MC + (c % 4) * 2048
            return dst[ds(32 * s, D), ds(col, MC)], ps[ds(32 * s, D), 0:MC]

        def qkv_repl(dst, h):
            # replicate strip-owned per-head blocks to the other 3 strips,
            # alternating issue queues (sync / gpsimd) to halve issue time
            qi = 0
            for s in range(4):
                for so in range(1, 4):
                    d_ = (s + so) % 4
                    eng = nc.sync if qi % 2 == 0 else nc.gpsimd
                    qi += 1
                    eng.dma_start(
                        dst[ds(32 * d_, D), ds(s * 2048 + h * 1024, 1024)],
                        dst[ds(32 * s, D), ds(s * 2048 + h * 1024, 1024)])

        with tc.tile_pool(name="p0psum", bufs=4, space="PSUM") as p0:
            for i in range(2):  # big chunks: DMA issue is serial per queue
                nc.sync.dma_start(x1_t[:, ts(i, N // 2)], x1[:, ts(i, N // 2)])
            for s in range(4):
                nc.gpsimd.dma_start(wp4_t[ds(32 * s, D), :], wp_t[:])
            ev = 0
            for w_t, dst in ((wk_t, ksp), (wq_t, qsp)):
                for c in range(N // MC):
                    ps = p0.tile([96 + D, MC], F32, tag="p0")
                    dst_ap, src_ap = qkv_mm(w_t, dst, 0, c, ps)
                    # alternate evac engine: latency is the phase-0 gate
                    if ev % 2 == 0:
                        nc.vector.tensor_copy(dst_ap, src_ap)
                    else:
                        nc.scalar.copy(dst_ap, src_ap)
                    ev += 1
                qkv_repl(dst, 0)

        # ---- phase 1: attention, software-pipelined ----
        with (
            tc.tile_pool(name="ep", bufs=14) as ep,
            tc.tile_pool(name="rp", bufs=4) as rp,
            tc.tile_pool(name="wnp", bufs=10) as wnp,
            tc.tile_pool(name="sapsum", bufs=2, space="PSUM") as sp,
            tc.tile_pool(name="avpsum", bufs=2, space="PSUM") as ap,
        ):
            def av_chain(sg_idx, chains, mc):
                """AV chain for m-chunk mc of supergroup sg_idx. Link j
                streams on PE column strip j%2 and accumulates into that
                partition half of a [128,512] PSUM tile, so consecutive
                links use disjoint PE cells / PSUM halves and pipeline.
                The two halves are partial sums; the host adds them."""
                yps = ap.tile([NT, MC], F32, tag="av", name=f"av{sg_idx}_{mc}")
                gl = len(chains)
                for j, (wn_t, e_t) in enumerate(chains):
                    strip = 64 * (j % 2)
                    nc.tensor.matmul(
                        yps[ds(strip, C), :], wn_t[:], e_t[:, ts(mc, MC)],
                        start=(j < 2), stop=(j >= gl - 2),
                        tile_position=(0, strip))
                dst = y_sb[:, ts(mc, MC)]
                if sg_idx == 0:
                    nc.vector.tensor_copy(dst, yps[:])
                else:
                    nc.vector.tensor_add(dst, dst, yps[:])
                if sg_idx == 2 * NSG - 1:  # final supergroup: stream out
                    nc.sync.dma_start(y[:, ts(mc, MC)], dst)

            state = dict(prev=None, prev_idx=-1, pend=[], cur=[], sgi=0)
            wn_queue = []  # (h, nt, rinv, e_t) awaiting JIT Wn emission

            # deferred qkv work (v both heads, k/q head 1), woven into the
            # early main-loop slots; PSUM borrowed from the av pool
            def misc(w_t, dst, h, c, repl):
                def emit():
                    ps = ap.tile([NT, MC], F32, tag="av",
                                 name=f"misc{h}_{c}")
                    dst_ap, src_ap = qkv_mm(w_t, dst, h, c, ps)
                    nc.vector.tensor_copy(dst_ap, src_ap)
                    if repl:
                        qkv_repl(dst, h)
                return emit
            pending_misc = []
            for w_t, dst, h in ((wv_t, v4_sb, 0), (wk_t, ksp, 1),
                                (wq_t, qsp, 1), (wv_t, v4_sb, 1)):
                for c in range(N // MC):
                    pending_misc.append(
                        misc(w_t, dst, h, c,
                             repl=(dst is not v4_sb and c == N // MC - 1)))

            def emit_wn():
                """JIT Wn for the oldest pending tile: (v^T wp) * rinv,
                PSUM borrowed from the av pool. Completes a supergroup
                when its 4th member lands."""
                h, nt, rinv, e_t = wn_queue.pop(0)
                c = nt // 4
                s = c % 4
                col = h * 1024 + (c // 4) * MC + (nt % 4) * NT
                wn_ps = ap.tile([NT, MC], F32, tag="av",
                                name=f"wnps{h}_{nt}")
                nc.tensor.matmul(
                    wn_ps[:, 0:C], v4_sb[ds(32 * s, D), ds(col, NT)],
                    wp4_t[ds(32 * s, D), ts(h, C)],
                    start=True, stop=True, tile_position=(32 * s, 0))
                wn_t = wnp.tile([NT, C], BF16, tag="wn", name=f"wn{h}_{nt}")
                nc.vector.tensor_scalar_mul(wn_t[:], wn_ps[:, 0:C], rinv[:])
                state["cur"].append((wn_t, e_t))
                if len(state["cur"]) == G:
                    while state["pend"]:  # unwoven chains (tail groups)
                        av_chain(state["prev_idx"], state["prev"],
                                 state["pend"].pop(0))
                    state["prev"], state["cur"] = state["cur"], []
                    state["prev_idx"] = state["sgi"]
                    state["sgi"] += 1
                    state["pend"] = list(range(MCN))

            mm_i = 0
            for h in range(2):
                for nt in range(NTILES):
                    e_t = ep.tile([NT, N], BF16, tag="e", name=f"e{h}_{nt}")
                    rsp = rp.tile([NT, len(SCH)], F32, tag="rs", name="rsp")
                    off = 0
                    qc = nt // 4
                    qcol = ((qc % 4) * 2048 + h * 1024 + (qc // 4) * MC
                            + (nt % 4) * NT)
                    for ci, csz in enumerate(SCH):
                        s_ps = sp.tile([NT, SCH[0]], F32, tag="sa",
                                       name="s_ps")
                        for i in range(csz // MC):
                            st = 32 * (mm_i % 4)
                            mm_i += 1
                            c = (off + i * MC) // MC
                            kcol = ((c % 4) * 2048 + h * 1024
                                    + (c // 4) * MC)
                            nc.tensor.matmul(
                                s_ps[:, ts(i, MC)],
                                qsp[ds(st, D), ds(qcol, NT)],
                                ksp[ds(st, D), ds(kcol, MC)],
                                start=True, stop=True, tile_position=(st, 0))
                        nc.scalar.activation(
                            e_t[:, ds(off, csz)], s_ps[:, :csz],
                            AF.Exp, accum_out=rsp[:, ds(ci, 1)])
                        off += csz
                        # c0 slot: JIT Wn of the previous tile (its rinv is
                        # ready by now). c1/c2 slots: weave prev-supergroup
                        # AV chains (keeps them off the path of the next
                        # tile's first score chunk). Deferred qkv work
                        # drains at 1-2 items per slot in the early tiles.
                        chained = False
                        if ci == 0:
                            if wn_queue:
                                emit_wn()
                        elif state["pend"]:
                            av_chain(state["prev_idx"], state["prev"],
                                     state["pend"].pop(0))
                            chained = True
                        if pending_misc and not chained:
                            pending_misc.pop(0)()
                    rinv = rp.tile([NT, 1], F32, tag="ri", name="rinv")
                    rs = rp.tile([NT, 1], F32, tag="r1", name="rs")
                    nc.vector.reduce_sum(
                        rs[:], rsp[:], axis=mybir.AxisListType.X)
                    nc.vector.reciprocal(rinv[:], rs[:])
                    wn_queue.append((h, nt, rinv, e_t))

            # ---- tail: flush pending Wn + the last supergroups' chains ----
            while wn_queue:
                while state["pend"]:
                    av_chain(state["prev_idx"], state["prev"],
                             state["pend"].pop(0))
                emit_wn()
            while state["pend"]:
                av_chain(state["prev_idx"], state["prev"],
                         state["pend"].pop(0))


_PROGRAM = None


def _get_program():
    global _PROGRAM
    if _PROGRAM is None:
        nc = bacc.Bacc("TRN2", target_bir_lowering=False, debug=False,
                       num_devices=8)
        x1 = nc.dram_tensor("x1", [C + 1, N], BF16, kind="ExternalInput").ap()
        wq = nc.dram_tensor("wq", [C + 1, 2 * D], BF16, kind="ExternalInput").ap()
        wk = nc.dram_tensor("wk", [C + 1, 2 * D], BF16, kind="ExternalInput").ap()
        wv = nc.dram_tensor("wv", [C + 1, 2 * D], BF16, kind="ExternalInput").ap()
        wp = nc.dram_tensor("wp", [D, 2 * C], BF16, kind="ExternalInput").ap()
        y = nc.dram_tensor("y", [NT, N], F32, kind="ExternalOutput").ap()
        with tile.TileContext(nc) as tc:
            _body(tc, y, x1, wq, wk, wv, wp)
        nc.compile()
        _PROGRAM = nc
    return _PROGRAM


def _make_in_maps(x, qkv_w, qkv_b, proj_w, proj_b=None):
    x = np.asarray(x, dtype=np.float32)
    qkv_w = np.asarray(qkv_w, dtype=np.float32)
    qkv_b = np.asarray(qkv_b, dtype=np.float32)
    proj_w = np.asarray(proj_w, dtype=np.float32)
    bf = ml_dtypes.bfloat16

    in_maps = []
    for core in range(8):
        b = core // 2
        h0 = 2 * (core % 2)
        heads = (h0, h0 + 1)
        x1 = np.concatenate(
            [x[b].reshape(C, N), np.ones((1, N), np.float32)], axis=0)

        def aug_qk(block):
            w = np.empty((C + 1, 2 * D), np.float32)
            for j, h in enumerate(heads):
                rows = slice(block * C + h * D, block * C + (h + 1) * D)
                w[:C, j * D:(j + 1) * D] = qkv_w[rows, :].T
                w[C, j * D:(j + 1) * D] = qkv_b[rows]
            return w.astype(bf)

        wp = np.concatenate(
            [np.ascontiguousarray(proj_w[:, h * D:(h + 1) * D].T)
             for h in heads], axis=1)  # [16, 128]

        in_maps.append({
            "x1": np.ascontiguousarray(x1.astype(bf)),
            "wq": aug_qk(0),
            "wk": aug_qk(1),
            "wv": aug_qk(2),
            "wp": np.ascontiguousarray(wp.astype(bf)),
        })
    return in_maps


def run_cores(inputs, **kw):
    """Compile+run on the 8 cores; returns BassKernelResults."""
    nc = _get_program()
    in_maps = _make_in_maps(**inputs)
    return run_bass_kernel_spmd(nc, in_maps, list(range(8)), **kw)


def kernel(x, qkv_w, qkv_b, proj_w, proj_b):
    res = run_cores(dict(x=x, qkv_w=qkv_w, qkv_b=qkv_b,
                         proj_w=proj_w, proj_b=proj_b))
    proj_b = np.asarray(proj_b, dtype=np.float32)
    parts = [r["y"] for r in res.results]  # [128, N]: two partial halves
    out = np.empty((B, C, N), np.float32)
    for b in range(B):
        p0, p1 = parts[2 * b], parts[2 * b + 1]
        out[b] = (p0[:C] + p0[C:]) + (p1[:C] + p1[C:]) + proj_b[:, None]
    return out.reshape(B, C, 64, 64)


if __name__ == "__main__":
    _get_program()
    print("program built OK")


# revision 34
# speedup vs baseline: 1.2049x; 1.2049x over previous
"""Trainium2 Bass kernel for nn_Attention_82540681494971.

Spatial self-attention block (LDM AttnBlock style, unscaled):
  qkv = conv1x1(x);  s = q^T k  [n x n] per (b,head);  attn = softmax(s, axis=-1)
  out[d,m] = sum_n v[d,n] attn[n,m];  y = conv1x1(out)

Shapes: B=4, C=64, H=W=64 -> n=4096 tokens, HEAD=4, d=16.

Sharding: 8 cores, core c handles batch b=c//2 and heads (0,1) if c%2==0
else (2,3). Each core computes a partial projection output over its two
heads' channels; host sums the two partials per batch and adds proj bias.

The kernel is ACT(exp)-bound: 2 heads x 4096 x 4096 exps per core at
1 elem/lane/cycle @ 1.2 GHz is ~220us. Everything else is organized to
keep the scalar engine streaming exp with minimal per-instruction
overhead and zero stalls:

- All-bf16 data path (inputs pre-cast on host). Scores s = q^T k with
  K=16 per head; error budget validated ~1.4e-3 vs gate 2e-2.
- Score matmuls rotate over 4 PE row strips (tile_position (32s, 0)) so
  LDWEIGHTS pulls ahead and up to 4 streams run concurrently.
- exp in 2 chunks of 2048 per n-tile (2 ACTIVATEs + 2 accumulator
  reads), double-buffered in ALL 8 PSUM banks ([128,2048] x 2).
- Projection is folded into AV: per n-tile, Wn = (v^T wp) * rinv
  [128 x 64] bf16; y[o,m] += Wn^T E chains with M=64. Chain PSUM lives
  *transiently* inside the score buffer that exp just drained (cols
  1536:2048, partition half 64*(mc%2)) - no dedicated AV banks needed.
- y partials accumulate in y_sb[128, 2048] (m-chunk mc at partition
  half mc%2, col block mc//2), DMA'd out per chunk at the end.
"""

import numpy as np
import ml_dtypes
from contextlib import ExitStack

import concourse.bass as bass
import concourse.mybir as mybir
import concourse.tile as tile
from concourse import bacc
from concourse.bass import ts, ds
from concourse.bass_utils import run_bass_kernel_spmd

F32 = mybir.dt.float32
BF16 = mybir.dt.bfloat16
AF = mybir.ActivationFunctionType

B, C, HEAD, D = 4, 64, 4, 16
N = 4096          # tokens = H*W
NT = 128          # n-tile (partition) size
NTILES = N // NT  # 32
MC = 512          # matmul free-dim chunk
MCN = N // MC     # 8 m-chunks
SCH = (1536, 1536, 1024)   # scores/exp PSUM chunking (2-buffer ring)
G = 4              # n-tiles per AV supergroup
NSG = NTILES // G  # supergroups per head


def _body(tc, y, x1, wq, wk, wv, wp):
    nc = tc.nc
    ctx = ExitStack()
    with ctx:
        pp = ctx.enter_context(tc.tile_pool(name="persist", bufs=1))
        cp = ctx.enter_context(tc.tile_pool(name="consts", bufs=1))

        # ---- constants (all bf16) ----
        wq_t = cp.tile([C + 1, 2 * D], BF16)
        wk_t = cp.tile([C + 1, 2 * D], BF16)
        wv_t = cp.tile([C + 1, 2 * D], BF16)
        wp_t = cp.tile([D, 2 * C], BF16)     # [16,128]: head0 cols 0-63, head1 64-127
        nc.gpsimd.dma_start(wq_t[:], wq[:])
        nc.gpsimd.dma_start(wk_t[:], wk[:])
        nc.gpsimd.dma_start(wv_t[:], wv[:])
        nc.gpsimd.dma_start(wp_t[:], wp[:])

        # ---- persistent SBUF ----
        # q/k replicated on 4 PE row strips (partitions 32s..32s+15),
        # head-major cols.
        qsp = pp.tile([96 + D, 2 * N], BF16)
        ksp = pp.tile([96 + D, 2 * N], BF16)
        v4_sb = pp.tile([96 + D, 2048], BF16)   # v, strip-local layout
        wp4_t = pp.tile([96 + D, 2 * C], BF16)  # wp replicated on 4 strips
        x1_t = pp.tile([C + 1, N], BF16)        # input (woven qkv reads it)
        # y partials: two partition-half partial sums (host adds them)
        y_sb = pp.tile([NT, N], F32)

        # ---- phase 0: k/q for head 0 ONLY (gates the first exp) ----
        # qsp/ksp use a strip-major column layout: chunk c of head h lives at
        # col4(h, c) = (c%4)*2048 + h*1024 + (c//4)*512. The (q|k, h, c)
        # matmul runs at PE column strip c%4 (output on PSUM partitions
        # 32s..32s+15) and evacuates straight into its own row strip of
        # qsp/ksp - no partition moves - then per-head [16,1024] DMAs
        # replicate each strip's block to the other strips. v and the
        # head-1 q/k are WOVEN into the main loop (see pending_misc).
        def qkv_mm(w_t, dst, h, c, ps):
            s = c % 4
            nc.tensor.matmul(
                ps[ds(32 * s, D), 0:MC], w_t[:, ts(h, D)],
                x1_t[:, ts(c, MC)],
                start=True, stop=True, tile_position=(0, 32 * s))
            if dst is v4_sb:
                col = h * 1024 + (c // 4) * MC
            else:
                col = h * 1024 + (c // 4) * MC + (c % 4) * 2048
            return dst[ds(32 * s, D), ds(col, MC)], ps[ds(32 * s, D), 0:MC]

        def qkv_repl(dst, h):
            # replicate strip-owned per-head blocks to the other 3 strips,
            # alternating issue queues (sync / gpsimd) to halve issue time
            qi = 0
            for s in range(4):
                for so in range(1, 4):
                    d_ = (s + so) % 4
                    eng = nc.sync if qi % 2 == 0 else nc.gpsimd
                    qi += 1
                    eng.dma_start(
                        dst[ds(32 * d_, D), ds(s * 2048 + h * 1024, 1024)],
                        dst[ds(32 * s, D), ds(s * 2048 + h * 1024, 1024)])

        with tc.tile_pool(name="p0psum", bufs=4, space="PSUM") as p0:
            for i in range(2):  # big chunks: DMA issue is serial per queue
                nc.sync.dma_start(x1_t[:, ts(i, N // 2)], x1[:, ts(i, N // 2)])
            for s in range(4):
                nc.gpsimd.dma_start(wp4_t[ds(32 * s, D), :], wp_t[:])
            # Chunk pairs (c, c+4) share a PE strip and are column-adjacent
            # in qsp/ksp, so each pair shares one 2-bank PSUM tile and
            # evacuates in a single [16,1024] copy - halves the evac count
            # that gates the first exp.
            ev = 0
            for w_t, dst in ((wk_t, ksp), (wq_t, qsp)):
                for cp_ in range(4):
                    s = cp_ % 4
                    ps = p0.tile([96 + D, 2 * MC], F32, tag="p0")
                    for half, c in enumerate((cp_, cp_ + 4)):
                        nc.tensor.matmul(
                            ps[ds(32 * s, D), ds(half * MC, MC)],
                            w_t[:, ts(0, D)], x1_t[:, ts(c, MC)],
                            start=True, stop=True,
                            tile_position=(0, 32 * s))
                    dst_ap = dst[ds(32 * s, D), ds(s * 2048, 2 * MC)]
                    # alternate evac engine: latency is the phase-0 gate
                    if ev % 2 == 0:
                        nc.vector.tensor_copy(dst_ap, ps[ds(32 * s, D), :])
                    else:
                        nc.scalar.copy(dst_ap, ps[ds(32 * s, D), :])
                    ev += 1
                qkv_repl(dst, 0)

        # ---- phase 1: attention, software-pipelined ----
        with (
            tc.tile_pool(name="ep", bufs=14) as ep,
            tc.tile_pool(name="rp", bufs=4) as rp,
            tc.tile_pool(name="wnp", bufs=10) as wnp,
            tc.tile_pool(name="sapsum", bufs=2, space="PSUM") as sp,
            tc.tile_pool(name="avpsum", bufs=2, space="PSUM") as ap,
        ):
            def av_chain(sg_idx, chains, mc):
                """AV chain for m-chunk mc of supergroup sg_idx. Link j
                streams on PE column strip j%2 and accumulates into that
                partition half of a [128,512] PSUM tile, so consecutive
                links use disjoint PE cells / PSUM halves and pipeline.
                The two halves are partial sums; the host adds them."""
                yps = ap.tile([NT, MC], F32, tag="av", name=f"av{sg_idx}_{mc}")
                gl = len(chains)
                for j, (wn_t, e_t) in enumerate(chains):
                    strip = 64 * (j % 2)
                    nc.tensor.matmul(
                        yps[ds(strip, C), :], wn_t[:], e_t[:, ts(mc, MC)],
                        start=(j < 2), stop=(j >= gl - 2),
                        tile_position=(0, strip))
                dst = y_sb[:, ts(mc, MC)]
                if sg_idx == 0:
                    nc.vector.tensor_copy(dst, yps[:])
                else:
                    nc.vector.tensor_add(dst, dst, yps[:])
                if sg_idx == 2 * NSG - 1:  # final supergroup: stream out
                    nc.sync.dma_start(y[:, ts(mc, MC)], dst)

            state = dict(prev=None, prev_idx=-1, pend=[], cur=[], sgi=0)
            wn_queue = []  # (h, nt, rinv, e_t) awaiting JIT Wn emission

            # deferred qkv work (v both heads, k/q head 1), woven into the
            # early main-loop slots; PSUM borrowed from the av pool
            def misc(w_t, dst, h, c, repl):
                def emit():
                    ps = ap.tile([NT, MC], F32, tag="av",
                                 name=f"misc{h}_{c}")
                    dst_ap, src_ap = qkv_mm(w_t, dst, h, c, ps)
                    nc.vector.tensor_copy(dst_ap, src_ap)
                    if repl:
                        qkv_repl(dst, h)
                return emit
            pending_misc = []
            for w_t, dst, h in ((wv_t, v4_sb, 0), (wk_t, ksp, 1),
                                (wq_t, qsp, 1), (wv_t, v4_sb, 1)):
                for c in range(N // MC):
                    pending_misc.append(
                        misc(w_t, dst, h, c,
                             repl=(dst is not v4_sb and c == N // MC - 1)))

            def emit_wn():
                """JIT Wn for the oldest pending tile: (v^T wp) * rinv,
                PSUM borrowed from the av pool. Completes a supergroup
                when its 4th member lands."""
                h, nt, rinv, e_t = wn_queue.pop(0)
                c = nt // 4
                s = c % 4
                col = h * 1024 + (c // 4) * MC + (nt % 4) * NT
                wn_ps = ap.tile([NT, MC], F32, tag="av",
                                name=f"wnps{h}_{nt}")
                nc.tensor.matmul(
                    wn_ps[:, 0:C], v4_sb[ds(32 * s, D), ds(col, NT)],
                    wp4_t[ds(32 * s, D), ts(h, C)],
                    start=True, stop=True, tile_position=(32 * s, 0))
                wn_t = wnp.tile([NT, C], BF16, tag="wn", name=f"wn{h}_{nt}")
                nc.vector.tensor_scalar_mul(wn_t[:], wn_ps[:, 0:C], rinv[:])
                state["cur"].append((wn_t, e_t))
                if len(state["cur"]) == G:
                    while state["pend"]:  # unwoven chains (tail groups)
                        av_chain(state["prev_idx"], state["prev"],
                                 state["pend"].pop(0))
                    state["prev"], state["cur"] = state["cur"], []
                    state["prev_idx"] = state["sgi"]
                    state["sgi"] += 1
                    state["pend"] = list(range(MCN))

            mm_i = 0
            for h in range(2):
                for nt in range(NTILES):
                    e_t = ep.tile([NT, N], BF16, tag="e", name=f"e{h}_{nt}")
                    rsp = rp.tile([NT, len(SCH)], F32, tag="rs", name="rsp")
                    off = 0
                    qc = nt // 4
                    qcol = ((qc % 4) * 2048 + h * 1024 + (qc // 4) * MC
                            + (nt % 4) * NT)
                    for ci, csz in enumerate(SCH):
                        s_ps = sp.tile([NT, SCH[0]], F32, tag="sa",
                                       name="s_ps")
                        for i in range(csz // MC):
                            st = 32 * (mm_i % 4)
                            mm_i += 1
                            c = (off + i * MC) // MC
                            kcol = ((c % 4) * 2048 + h * 1024
                                    + (c // 4) * MC)
                            nc.tensor.matmul(
                                s_ps[:, ts(i, MC)],
                                qsp[ds(st, D), ds(qcol, NT)],
                                ksp[ds(st, D), ds(kcol, MC)],
                                start=True, stop=True, tile_position=(st, 0))
                        nc.scalar.activation(
                            e_t[:, ds(off, csz)], s_ps[:, :csz],
                            AF.Exp, accum_out=rsp[:, ds(ci, 1)])
                        off += csz
                        # c0 slot: JIT Wn of the previous tile (its rinv is
                        # ready by now). c1/c2 slots: weave prev-supergroup
                        # AV chains (keeps them off the path of the next
                        # tile's first score chunk). Deferred qkv work
                        # drains at 1-2 items per slot in the early tiles.
                        chained = False
                        if ci == 0:
                            if wn_queue:
                                emit_wn()
                        elif state["pend"]:
                            av_chain(state["prev_idx"], state["prev"],
                                     state["pend"].pop(0))
                            chained = True
                        if pending_misc and not chained:
                            pending_misc.pop(0)()
                    rinv = rp.tile([NT, 1], F32, tag="ri", name="rinv")
                    rs = rp.tile([NT, 1], F32, tag="r1", name="rs")
                    nc.vector.reduce_sum(
                        rs[:], rsp[:], axis=mybir.AxisListType.X)
                    nc.vector.reciprocal(rinv[:], rs[:])
                    wn_queue.append((h, nt, rinv, e_t))

            # ---- tail: flush pending Wn + the last supergroups' chains ----
            while wn_queue:
                while state["pend"]:
                    av_chain(state["prev_idx"], state["prev"],
                             state["pend"].pop(0))
                emit_wn()
            while state["pend"]:
                av_chain(state["prev_idx"], state["prev"],
                         state["pend"].pop(0))


_PROGRAM = None


def _get_program():
    global _PROGRAM
    if _PROGRAM is None:
        nc = bacc.Bacc("TRN2", target_bir_lowering=False, debug=False,
                       num_devices=8)
        x1 = nc.dram_tensor("x1", [C + 1, N], BF16, kind="ExternalInput").ap()
        wq = nc.dram_tensor("wq", [C + 1, 2 * D], BF16, kind="ExternalInput").ap()
        wk = nc.dram_tensor("wk", [C + 1, 2 * D], BF16, kind="ExternalInput").ap()
        wv = nc.dram_tensor("wv", [C + 1, 2 * D], BF16, kind="ExternalInput").ap()
        wp = nc.dram_tensor("wp", [D, 2 * C], BF16, kind="ExternalInput").ap()
        y = nc.dram_tensor("y", [NT, N], F32, kind="ExternalOutput").ap()
        with tile.TileContext(nc) as tc:
            _body(tc, y, x1, wq, wk, wv, wp)
        nc.compile()
        _PROGRAM = nc
    return _PROGRAM


def _make_in_maps(x, qkv_w, qkv_b, proj_w, proj_b=None):
    x = np.asarray(x, dtype=np.float32)
    qkv_w = np.asarray(qkv_w, dtype=np.float32)
    qkv_b = np.asarray(qkv_b, dtype=np.float32)
    proj_w = np.asarray(proj_w, dtype=np.float32)
    bf = ml_dtypes.bfloat16

    in_maps = []
    for core in range(8):
        b = core // 2
        h0 = 2 * (core % 2)
        heads = (h0, h0 + 1)
        x1 = np.concatenate(
            [x[b].reshape(C, N), np.ones((1, N), np.float32)], axis=0)

        def aug_qk(block):
            w = np.empty((C + 1, 2 * D), np.float32)
            for j, h in enumerate(heads):
                rows = slice(block * C + h * D, block * C + (h + 1) * D)
                w[:C, j * D:(j + 1) * D] = qkv_w[rows, :].T
                w[C, j * D:(j + 1) * D] = qkv_b[rows]
            return w.astype(bf)

        wp = np.concatenate(
            [np.ascontiguousarray(proj_w[:, h * D:(h + 1) * D].T)
             for h in heads], axis=1)  # [16, 128]

        in_maps.append({
            "x1": np.ascontiguousarray(x1.astype(bf)),
            "wq": aug_qk(0),
            "wk": aug_qk(1),
            "wv": aug_qk(2),
            "wp": np.ascontiguousarray(wp.astype(bf)),
        })
    return in_maps


def run_cores(inputs, **kw):
    """Compile+run on the 8 cores; returns BassKernelResults."""
    nc = _get_program()
    in_maps = _make_in_maps(**inputs)
    return run_bass_kernel_spmd(nc, in_maps, list(range(8)), **kw)


def kernel(x, qkv_w, qkv_b, proj_w, proj_b):
    res = run_cores(dict(x=x, qkv_w=qkv_w, qkv_b=qkv_b,
                         proj_w=proj_w, proj_b=proj_b))
    proj_b = np.asarray(proj_b, dtype=np.float32)
    parts = [r["y"] for r in res.results]  # [128, N]: two partial halves
    out = np.empty((B, C, N), np.float32)
    for b in range(B):
        p0, p1 = parts[2 * b], parts[2 * b + 1]
        out[b] = (p0[:C] + p0[C:]) + (p1[:C] + p1[C:]) + proj_b[:, None]
    return out.reshape(B, C, 64, 64)


if __name__ == "__main__":
    _get_program()
    print("program built OK")
